# revision 3
# baseline (speedup 1.0000x reference)
"""GAT v2: layer-1 via host-pregathered x + PE matmuls (zero device gathers),
layer-2 via dma_gather on a 16-node-packed table2 view.

 - Same dst-partitioned slab structure as baseline (fid, K_t shared by all
   cores -> one SPMD program). Self-loop pinned to slab 0 (gives a_d).
 - Phase A: per slab k of tile t, h-row of every edge src is computed as
   xe_slab[256,128] @ W1aug (2 accumulating matmuls into PSUM), where xe is
   x gathered into edge-slab order ON THE HOST. Dummy slots use an x row
   solved (lstsq) to make alpha_src = -1e4 -> exp weight exactly 0.
 - Phase B: AllGather of compact t2l [12672,4] f32 -> table2 [101376,4].
 - Phase C: per tile, dma_gather (mlp Q7 library) fetches 256B rows of the
   [6336, 64] f32 view of table2 (16 nodes per row); per-slot 16-candidate
   bias mask (host-prepared, 0 or -1e4) selects the right node inside the
   softmax algebra. GpSimd ring drained every 8 gathers (reclaim is broken
   on this build; <=1024 idx per gather).
"""
import os
import numpy as np

N = 100000
E = 1600000
IN_DIM = 256
HID = 128
HEADS = 4
C1 = HID // HEADS
OUT = 2
NEG = 0.2
P = 128
NCORES = 8
N_ST_DATA = 784                   # 8 cores x 98 data tiles
T_PER_CORE = N_ST_DATA // NCORES + 1   # 99 (incl. all-dummy tail block)
T_DATA = T_PER_CORE - 1           # 98
N_ST = T_PER_CORE * NCORES        # 792
N_PAD = N_ST * P                  # 101376
ROWS_PER_CORE = T_PER_CORE * P    # 12672
PAD_ALPHA = -1.0e4
D1 = HID + 2 * HEADS              # 136
G16 = N_PAD // 16                 # 6336 16-node rows in table2 view
CHUNK = 8                         # slabs per dma_gather (<=1024 idx)
DRAIN_EVERY = 14                   # gathers per gpsimd drain


def _host_prep(x, edge_index, W1, att_src1, att_dst1, b1, W2, att_src2, att_dst2, b2):
    import ml_dtypes
    bf16 = ml_dtypes.bfloat16
    src0 = np.asarray(edge_index[0], dtype=np.int64)
    dst0 = np.asarray(edge_index[1], dtype=np.int64)
    loops = np.arange(N, dtype=np.int64)
    src = np.concatenate([src0, loops])
    dst = np.concatenate([dst0, loops])

    deg = np.bincount(dst, minlength=N)
    order = np.argsort(deg, kind="stable")        # sorted position -> old id

    p_ar = np.arange(N_ST_DATA * P, dtype=np.int64)
    s_ar = p_ar // P
    d_ar = p_ar % P
    t_ar = s_ar // NCORES
    c_ar = s_ar % NCORES
    fid_of_sorted = (c_ar * T_PER_CORE + t_ar) * P + d_ar

    fid_of_old = np.empty(N, dtype=np.int64)
    fid_of_old[order] = fid_of_sorted[:N]
    old_of_fid = np.full(N_PAD, -1, dtype=np.int64)
    old_of_fid[fid_of_old] = np.arange(N)

    nsrc = fid_of_old[src]
    ndst = fid_of_old[dst]
    ndeg = np.zeros(N_PAD, dtype=np.int64)
    ndeg[fid_of_old] = deg

    K_ct = ndeg.reshape(NCORES, T_PER_CORE, P).max(-1)
    K_t = np.maximum(K_ct.max(0), 1)[:T_DATA]               # [98]
    tot_slabs = int(K_t.sum())
    slab_off = np.concatenate([[0], np.cumsum(K_t)]).astype(np.int64)

    # slot index k of each edge within its dst; self-loop forced to k=0
    notself = (nsrc != ndst).astype(np.int8)
    key = np.lexsort((notself, ndst))
    ds_ = ndst[key]
    ss_ = nsrc[key]
    counts = np.bincount(ds_, minlength=N_PAD)
    run_start = np.zeros(N_PAD + 1, dtype=np.int64)
    run_start[1:] = np.cumsum(counts)
    kk = np.arange(ds_.size, dtype=np.int64) - run_start[ds_]

    blk = ds_ // P
    core = blk // T_PER_CORE
    t_idx = blk % T_PER_CORE
    d_rel = ds_ % P
    col = slab_off[t_idx] + kk

    # per-core slab tables: old x-row id of src (N = dummy), src fid (-1 = dummy)
    src_old = np.full((NCORES, P, tot_slabs), N, dtype=np.int64)
    src_old[core, d_rel, col] = old_of_fid[ss_]
    src_fid = np.full((NCORES, P, tot_slabs), -1, dtype=np.int64)
    src_fid[core, d_rel, col] = ss_

    # dummy x row: x_d @ (W1 @ A1s) == -1e4 for all 4 heads (min-norm lstsq)
    A1s = np.zeros((HID, HEADS), dtype=np.float64)
    A1d = np.zeros((HID, HEADS), dtype=np.float64)
    for h in range(HEADS):
        A1s[h * C1:(h + 1) * C1, h] = np.asarray(att_src1, np.float64)[h]
        A1d[h * C1:(h + 1) * C1, h] = np.asarray(att_dst1, np.float64)[h]
    W1_ = np.asarray(W1, np.float64)
    M = W1_ @ A1s                                           # [256, 4]
    x_dummy, *_ = np.linalg.lstsq(M.T, np.full(HEADS, PAD_ALPHA), rcond=None)

    x_pad = np.concatenate(
        [np.asarray(x, np.float32), x_dummy[None, :].astype(np.float32)], axis=0
    ).astype(bf16)                                          # [N+1, 256]

    # per-core xe: [128, tot_slabs * 2 * 128] bf16, xe[f, (s,i,d)] =
    # x_pad[src_old[c, d, s], i*128 + f]
    xe_cores = []
    for c in range(NCORES):
        srcs_T = src_old[c].T                               # [S, 128]
        xv = x_pad[srcs_T]                                  # [S, 128d, 256f]
        arr = np.ascontiguousarray(xv.transpose(2, 0, 1))   # [256, S, 128]
        xe = np.ascontiguousarray(
            arr.reshape(2, P, tot_slabs, P).transpose(1, 2, 0, 3)
        ).reshape(P, tot_slabs * 2 * P)
        xe_cores.append(xe)

    # phase-3 idx (wrapped int16, replicated in all 8 16-partition groups)
    # + bias mask [128, tot_slabs*16] bf16
    g16 = np.where(src_fid >= 0, src_fid >> 4, 0).astype(np.int16)   # [NC,128,S]
    sub = np.where(src_fid >= 0, src_fid & 15, -1)                   # [NC,128,S]
    idx3_cores = []
    bias_cores = []
    chunks = []                       # (tile, k0, k1, idx_col_off) shared
    icol = 0
    for t in range(T_DATA):
        K = int(K_t[t])
        off = int(slab_off[t])
        for k0 in range(0, K, CHUNK):
            k1 = min(k0 + CHUNK, K)
            chunks.append((t, k0, k1, icol))
            icol += (k1 - k0) * P // 16
    idx_cols_total = icol
    for c in range(NCORES):
        idx3 = np.zeros((P, idx_cols_total), dtype=np.int16)
        for (t, k0, k1, ic) in chunks:
            off = int(slab_off[t])
            flat = g16[c][:, off + k0:off + k1].T.reshape(-1)  # j=(k_rel)*128+d
            blkw = flat.reshape(-1, 16).T                      # [16, n/16]
            idx3[:, ic:ic + blkw.shape[1]] = np.tile(blkw, (8, 1))
        idx3_cores.append(idx3)
        b3 = np.full((P, tot_slabs, 16), PAD_ALPHA, dtype=np.float32)
        valid = sub[c] >= 0
        dd, ss2 = np.nonzero(valid)
        b3[dd, ss2, sub[c][dd, ss2]] = 0.0
        bias_cores.append(b3.reshape(P, tot_slabs * 16).astype(bf16))

    W1aug = np.concatenate(
        [W1_, W1_ @ A1s, W1_ @ A1d], axis=1).astype(np.float32)    # [256,136]
    W1aug_t = np.ascontiguousarray(
        W1aug.reshape(2, 128, D1).transpose(1, 0, 2)).astype(bf16)

    W2_ = np.asarray(W2, np.float32)
    a_s2 = np.asarray(att_src2, np.float32).reshape(OUT, 1)
    a_d2 = np.asarray(att_dst2, np.float32).reshape(OUT, 1)
    W2aug = np.concatenate([W2_, W2_ @ a_s2, W2_ @ a_d2], axis=1).astype(bf16)

    b1_b = np.tile(np.asarray(b1, np.float32)[None, :], (P, 1))
    b2_b = np.tile(np.asarray(b2, np.float32)[None, :], (P, 1))

    return dict(
        xe_cores=xe_cores, idx3_cores=idx3_cores, bias_cores=bias_cores,
        W1aug_t=W1aug_t, W2aug=W2aug, b1_b=b1_b, b2_b=b2_b,
        K_t=K_t, slab_off=slab_off, tot_slabs=tot_slabs, chunks=chunks,
        idx_cols_total=idx_cols_total, old_of_fid=old_of_fid,
    )


def _build_program(K_t, slab_off, tot_slabs, chunks, idx_cols_total,
                   dump_tables=False):
    import concourse.bass as bass
    import concourse.mybir as mybir
    import bass_rust as _bass_rust
    from concourse import library_config
    from concourse.tile import TileContext
    from concourse.masks import make_identity

    f32 = mybir.dt.float32
    bf16 = mybir.dt.bfloat16
    i16 = mybir.dt.int16
    AF = mybir.ActivationFunctionType
    OP = mybir.AluOpType

    nc = bass.Bass(target_bir_lowering=False)

    xe_in = nc.dram_tensor("xe", [P, tot_slabs * 2 * P], bf16, kind="ExternalInput")
    idx3_in = nc.dram_tensor("idx3", [P, idx_cols_total], i16, kind="ExternalInput")
    bias_in = nc.dram_tensor("bias3", [P, tot_slabs * 16], bf16, kind="ExternalInput")
    w1aug = nc.dram_tensor("w1aug", [P, 2, D1], bf16, kind="ExternalInput")
    w2aug = nc.dram_tensor("w2aug", [HID, 4], bf16, kind="ExternalInput")
    b1_b = nc.dram_tensor("b1_b", [P, HID], f32, kind="ExternalInput")
    b2_b = nc.dram_tensor("b2_b", [P, OUT], f32, kind="ExternalInput")
    out2 = nc.dram_tensor("out2", [ROWS_PER_CORE, OUT], f32, kind="ExternalOutput")
    if dump_tables:
        t2dump = nc.dram_tensor("t2dump", [N_PAD, 4], f32, kind="ExternalOutput")

    with TileContext(nc) as tc:
        with tc.tile_pool(name="dram", bufs=1, space="DRAM") as dpool, \
             tc.tile_pool(name="const", bufs=1) as cpool, \
             tc.tile_pool(name="xp", bufs=3) as xp, \
             tc.tile_pool(name="gat", bufs=2) as gat, \
             tc.tile_pool(name="sb", bufs=4) as sb, \
             tc.tile_pool(name="acc", bufs=2) as accp, \
             tc.tile_pool(name="g3", bufs=4) as g3p, \
             tc.tile_pool(name="ps", bufs=4, space="PSUM") as ps, \
             tc.tile_pool(name="ps2", bufs=2, space="PSUM") as ps2:

            t2l = dpool.tile([ROWS_PER_CORE, 4], f32)
            table2 = dpool.tile([N_PAD, 4], f32, addr_space="Shared")

            w1_sb = cpool.tile([P, 2, D1], bf16)
            nc.sync.dma_start(out=w1_sb[:], in_=w1aug[:, :, :])
            w2_sb = cpool.tile([HID, 4], bf16)
            nc.sync.dma_start(out=w2_sb[:], in_=w2aug[:, :])
            b1_sb = cpool.tile([P, HID], f32)
            nc.sync.dma_start(out=b1_sb[:], in_=b1_b[:, :])
            b2_sb = cpool.tile([P, OUT], f32)
            nc.sync.dma_start(out=b2_sb[:], in_=b2_b[:, :])
            ident = cpool.tile([P, P], bf16)
            make_identity(nc, ident[:])
            ix3 = cpool.tile([P, idx_cols_total], i16)
            nc.sync.dma_start(out=ix3[:], in_=idx3_in[:, :])
            ad2_all = cpool.tile([P, T_DATA], f32)

            # ---- Phase A: fused layer-1 per dst tile ----
            for t in range(T_DATA):
                K = int(K_t[t])
                off = int(slab_off[t])
                xt = xp.tile([P, K, 2, P], bf16, tag="xt")
                nc.sync.dma_start(
                    out=xt[:], in_=xe_in[:, off * 2 * P:(off + K) * 2 * P]
                    .rearrange("p (k i d) -> p k i d", i=2, d=P))
                gall = gat.tile([P, K, D1], bf16, tag="gall")
                for k in range(K):
                    hp = ps.tile([P, D1], f32, space="PSUM", tag="hp")
                    for i in range(2):
                        nc.tensor.matmul(out=hp[:], lhsT=xt[:, k, i, :],
                                         rhs=w1_sb[:, i, :],
                                         start=(i == 0), stop=(i == 1))
                    nc.scalar.activation(gall[:, k, :], hp[:], AF.Copy)
                # e[p, h, k] = a_src(slab k) + a_dst(slab 0 self-loop)
                w_all = sb.tile([P, HEADS, K], f32, tag="w_all")
                nc.vector.tensor_tensor(
                    out=w_all[:],
                    in0=gall[:, :, HID:HID + HEADS].rearrange("p k h -> p h k"),
                    in1=gall[:, 0, HID + HEADS:HID + 2 * HEADS].to_broadcast(
                        [P, HEADS, K]),
                    op=OP.add)
                nc.scalar.activation(w_all[:], w_all[:], AF.Prelu, alpha=NEG)
                w16 = sb.tile([P, HEADS, K], bf16, tag="w16")
                nc.scalar.activation(w16[:], w_all[:], AF.Exp)
                den = sb.tile([P, HEADS], f32, tag="den")
                nc.vector.tensor_reduce(out=den[:], in_=w16[:],
                                        axis=mybir.AxisListType.X, op=OP.add)
                rden = sb.tile([P, HEADS], f32, tag="rden")
                nc.vector.tensor_scalar(out=rden[:], in0=den[:], scalar1=1e-30,
                                        scalar2=None, op0=OP.add)
                nc.vector.reciprocal(rden[:], rden[:])
                u_all = accp.tile([P, K, HID], bf16, tag="u_all")
                nc.vector.tensor_tensor(
                    out=u_all[:].rearrange("p k (h c) -> p k h c", h=HEADS),
                    in0=gall[:, :, 0:HID].rearrange("p k (h c) -> p k h c", h=HEADS),
                    in1=w16[:].rearrange("p h k -> p k h").to_broadcast(
                        [P, K, HEADS, C1]),
                    op=OP.mult)
                acc = sb.tile([P, HID], f32, tag="acc")
                nc.vector.tensor_reduce(
                    out=acc[:], in_=u_all[:].rearrange("p k f -> p f k"),
                    axis=mybir.AxisListType.X, op=OP.add)
                h1 = sb.tile([P, HID], f32, tag="h1")
                nc.vector.tensor_tensor(
                    out=h1[:].rearrange("p (h c) -> p h c", h=HEADS),
                    in0=acc[:].rearrange("p (h c) -> p h c", h=HEADS),
                    in1=rden[:].to_broadcast([P, HEADS, C1]),
                    op=OP.mult)
                nc.vector.tensor_add(h1[:], h1[:], b1_sb[:])
                # elu
                m = sb.tile([P, HID], f32, tag="m")
                nc.vector.tensor_scalar(out=m[:], in0=h1[:], scalar1=0.0,
                                        scalar2=None, op0=OP.min)
                em = sb.tile([P, HID], f32, tag="em")
                nc.scalar.activation(em[:], m[:], AF.Exp)
                nc.vector.tensor_scalar(out=h1[:], in0=h1[:], scalar1=0.0,
                                        scalar2=None, op0=OP.max)
                nc.vector.tensor_add(h1[:], h1[:], em[:])
                h1b = sb.tile([P, HID], bf16, tag="h1b")
                nc.vector.tensor_scalar(out=h1b[:], in0=h1[:], scalar1=1.0,
                                        scalar2=None, op0=OP.subtract)
                # t2 row block = h1 @ W2aug (via PE transpose)
                h1tp = ps2.tile([P, P], bf16, space="PSUM", tag="h1tp")
                nc.tensor.transpose(out=h1tp[:], in_=h1b[:], identity=ident[:])
                h1t = sb.tile([P, P], bf16, tag="h1t")
                nc.vector.tensor_copy(h1t[:], h1tp[:])
                t2p = ps2.tile([P, 4], f32, space="PSUM", tag="t2p")
                nc.tensor.matmul(out=t2p[:], lhsT=h1t[:], rhs=w2_sb[:],
                                 start=True, stop=True)
                t2 = sb.tile([P, 4], f32, tag="t2")
                nc.vector.tensor_copy(t2[:], t2p[:])
                nc.vector.tensor_copy(ad2_all[:, t:t + 1], t2[:, 3:4])
                nc.scalar.dma_start(out=t2l[t * P:(t + 1) * P, :], in_=t2[:])

            # ---- Phase B: AllGather compact table2 ----
            nc.gpsimd.collective_compute(
                "AllGather", mybir.AluOpType.bypass,
                replica_groups=[list(range(NCORES))],
                ins=[t2l[:, :]], outs=[table2[:, :]])

            table16 = table2[:, :].rearrange("(a b) f -> a (b f)", b=16)

            regs = {}
            def reg_of(v):
                if v not in regs:
                    regs[v] = nc.gpsimd.to_reg(v)
                return regs[v]

            # ---- Phase C: layer-2 per dst tile ----
            gcount = 0
            cur = 0
            for t in range(T_DATA):
                K = int(K_t[t])
                off = int(slab_off[t])
                g3 = g3p.tile([P, K, 64], f32, tag="g3")
                while cur < len(chunks) and chunks[cur][0] == t:
                    _, k0, k1, ic = chunks[cur]
                    n_idx = (k1 - k0) * P
                    nc.gpsimd.dma_gather(
                        g3[:, k0:k1, :], table16,
                        ix3[:, ic:ic + n_idx // 16],
                        n_idx, reg_of(n_idx), 64)
                    gcount += 1
                    if gcount % DRAIN_EVERY == 0:
                        nc.gpsimd.drain()
                    cur += 1
                bias = sb.tile([P, K * 16], bf16, tag="bias3")
                nc.sync.dma_start(out=bias[:],
                                  in_=bias_in[:, off * 16:(off + K) * 16])
                e3 = sb.tile([P, K * 16], f32, tag="e3")
                nc.vector.tensor_tensor(
                    out=e3[:],
                    in0=g3[:].rearrange("p k (q v) -> p (k q) v", v=4)[:, :, 2]
                    .rearrange("p m -> p m"),
                    in1=bias[:],
                    op=OP.add)
                nc.vector.tensor_tensor(
                    out=e3[:], in0=e3[:],
                    in1=ad2_all[:, t:t + 1].to_broadcast([P, K * 16]),
                    op=OP.add)
                nc.scalar.activation(e3[:], e3[:], AF.Prelu, alpha=NEG)
                nc.scalar.activation(e3[:], e3[:], AF.Exp)
                den3 = sb.tile([P, 1], f32, tag="den3")
                nc.vector.tensor_reduce(out=den3[:], in_=e3[:],
                                        axis=mybir.AxisListType.X, op=OP.add)
                acc3 = sb.tile([P, OUT], f32, tag="acc3")
                u3 = sb.tile([P, K * 16], f32, tag="u3")
                for o in range(OUT):
                    nc.vector.tensor_tensor(
                        out=u3[:],
                        in0=g3[:].rearrange("p k (q v) -> p (k q) v", v=4)[:, :, o]
                        .rearrange("p m -> p m"),
                        in1=e3[:], op=OP.mult)
                    nc.vector.tensor_reduce(out=acc3[:, o:o + 1], in_=u3[:],
                                            axis=mybir.AxisListType.X, op=OP.add)
                rden3 = sb.tile([P, 1], f32, tag="rden3")
                nc.vector.tensor_scalar(out=rden3[:], in0=den3[:], scalar1=1e-30,
                                        scalar2=None, op0=OP.add)
                nc.vector.reciprocal(rden3[:], rden3[:])
                o2 = sb.tile([P, OUT], f32, tag="o2")
                nc.vector.tensor_tensor(out=o2[:], in0=acc3[:],
                                        in1=rden3[:].to_broadcast([P, OUT]),
                                        op=OP.mult)
                nc.vector.tensor_add(o2[:], o2[:], b2_sb[:])
                nc.sync.dma_start(out=out2[t * P:(t + 1) * P, :], in_=o2[:])

            if dump_tables:
                for r0 in range(0, N_PAD, 16896):
                    r1 = min(r0 + 16896, N_PAD)
                    nc.sync.dma_start(out=t2dump[r0:r1, :], in_=table2[r0:r1, :])

    # lower library loads + custom isa, then fix multi-wait instructions
    inst_type_to_lib_mask = {}
    for lib in library_config.all_libraries:
        for it in lib.instructions:
            inst_type_to_lib_mask[it] = inst_type_to_lib_mask.get(it, 0) | (
                1 << lib.index)
    _bass_rust.insert_library_loads(nc, inst_type_to_lib_mask,
                                    len(library_config.all_libraries),
                                    library_config.standard.index)
    mybir.codegen_inst_isa_subclasses(nc)
    _split_waits(nc)
    return nc


def _split_waits(nc, max_waits=1):
    """This walrus build allows one sync-wait slot per instruction; hoist
    excess waits onto same-engine InstNoOp carriers inserted just before."""
    import concourse.mybir as mybir
    ctr = 0
    for fn in nc.m.functions:
        for bb in fn.blocks:
            out = []
            changed = False
            for inst in bb.instructions:
                si = inst.sync_info
                if si is not None and len(si.on_wait) > max_waits:
                    waits = list(si.on_wait)
                    extra, keep = waits[:-max_waits], waits[-max_waits:]
                    for i in range(0, len(extra), max_waits):
                        ctr += 1
                        nop = mybir.InstNoOp(name=f"waitfix-{ctr}", ins=[], outs=[])
                        nop.engine = inst.engine
                        nop.sync_info = mybir.SyncInfo(
                            on_wait=extra[i:i + max_waits], on_update=[])
                        out.append(nop)
                    si.on_wait = keep
                    inst.sync_info = si
                    changed = True
                out.append(inst)
            if changed:
                bb.instructions = out
    return ctr


def kernel(x, edge_index, W1, att_src1, att_dst1, b1, W2, att_src2, att_dst2, b2):
    from concourse.bass_utils import run_bass_kernel_spmd

    prep = _host_prep(x, edge_index, W1, att_src1, att_dst1, b1,
                      W2, att_src2, att_dst2, b2)
    dump = bool(os.environ.get("GAT_DUMP"))
    nc = _build_program(prep["K_t"], prep["slab_off"], prep["tot_slabs"],
                        prep["chunks"], prep["idx_cols_total"],
                        dump_tables=dump)

    in_maps = []
    for c in range(NCORES):
        in_maps.append({
            "xe": prep["xe_cores"][c],
            "idx3": prep["idx3_cores"][c],
            "bias3": prep["bias_cores"][c],
            "w1aug": prep["W1aug_t"],
            "w2aug": prep["W2aug"],
            "b1_b": prep["b1_b"],
            "b2_b": prep["b2_b"],
        })

    trace = bool(os.environ.get("GAT_TRACE"))
    if trace:
        _install_ntff_shim()
    r = run_bass_kernel_spmd(nc, in_maps, core_ids=list(range(NCORES)),
                             trace=trace)
    if trace and r.exec_time_ns:
        print(f"HW exec time: {r.exec_time_ns} ns")

    old_of_fid = prep["old_of_fid"]
    out = np.zeros((N, OUT), dtype=np.float32)
    for c in range(NCORES):
        fid0 = c * ROWS_PER_CORE
        olds = old_of_fid[fid0:fid0 + ROWS_PER_CORE]
        m = olds >= 0
        out[olds[m]] = r.results[c]["out2"][m]
    if dump:
        np.save("/tmp/gat_t2dump.npy", r.results[0]["t2dump"])
        np.save("/tmp/gat_oldfid.npy", old_of_fid)
    return out


def _install_ntff_shim():
    """The image's antenv lacks axon_hooks; recreate it so trace=True works."""
    import sys, types
    if "antenv.axon_hooks" in sys.modules:
        return
    sys.path.insert(0, "/root/.axon_site/trn_agent_boot")
    try:
        import trn_boot
        hook = trn_boot._ntff_profile_via_ctypes("/opt/axon/libaxon_pjrt.so")
    except Exception:
        hook = None
    mod = types.ModuleType("antenv.axon_hooks")
    mod.get_axon_ntff_profile_hook = lambda: hook
    mod.set_axon_ntff_profile_hook = lambda h: None
    sys.modules["antenv.axon_hooks"] = mod


# revision 5
# speedup vs baseline: 1.0531x; 1.0531x over previous
"""GAT v2: layer-1 via host-pregathered x + PE matmuls (zero device gathers),
layer-2 via dma_gather on a 16-node-packed table2 view.

 - Same dst-partitioned slab structure as baseline (fid, K_t shared by all
   cores -> one SPMD program). Self-loop pinned to slab 0 (gives a_d).
 - Phase A: per slab k of tile t, h-row of every edge src is computed as
   xe_slab[256,128] @ W1aug (2 accumulating matmuls into PSUM), where xe is
   x gathered into edge-slab order ON THE HOST. Dummy slots use an x row
   solved (lstsq) to make alpha_src = -1e4 -> exp weight exactly 0.
 - Phase B: AllGather of compact t2l [12672,4] f32 -> table2 [101376,4].
 - Phase C: per tile, dma_gather (mlp Q7 library) fetches 256B rows of the
   [6336, 64] f32 view of table2 (16 nodes per row); per-slot 16-candidate
   bias mask (host-prepared, 0 or -1e4) selects the right node inside the
   softmax algebra. GpSimd ring drained every 8 gathers (reclaim is broken
   on this build; <=1024 idx per gather).
"""
import os
import numpy as np

N = 100000
E = 1600000
IN_DIM = 256
HID = 128
HEADS = 4
C1 = HID // HEADS
OUT = 2
NEG = 0.2
P = 128
NCORES = 8
N_ST_DATA = 784                   # 8 cores x 98 data tiles
T_PER_CORE = N_ST_DATA // NCORES + 1   # 99 (incl. all-dummy tail block)
T_DATA = T_PER_CORE - 1           # 98
N_ST = T_PER_CORE * NCORES        # 792
N_PAD = N_ST * P                  # 101376
ROWS_PER_CORE = T_PER_CORE * P    # 12672
PAD_ALPHA = -1.0e4
D1 = HID + 2 * HEADS              # 136
G16 = N_PAD // 16                 # 6336 16-node rows in table2 view
CHUNK = 8                         # slabs per dma_gather (<=1024 idx)
DRAIN_EVERY = 14                   # gathers per gpsimd drain


def _host_prep(x, edge_index, W1, att_src1, att_dst1, b1, W2, att_src2, att_dst2, b2):
    import ml_dtypes
    bf16 = ml_dtypes.bfloat16
    src0 = np.asarray(edge_index[0], dtype=np.int64)
    dst0 = np.asarray(edge_index[1], dtype=np.int64)
    loops = np.arange(N, dtype=np.int64)
    src = np.concatenate([src0, loops])
    dst = np.concatenate([dst0, loops])

    deg = np.bincount(dst, minlength=N)
    order = np.argsort(deg, kind="stable")        # sorted position -> old id

    p_ar = np.arange(N_ST_DATA * P, dtype=np.int64)
    s_ar = p_ar // P
    d_ar = p_ar % P
    t_ar = s_ar // NCORES
    c_ar = s_ar % NCORES
    fid_of_sorted = (c_ar * T_PER_CORE + t_ar) * P + d_ar

    fid_of_old = np.empty(N, dtype=np.int64)
    fid_of_old[order] = fid_of_sorted[:N]
    old_of_fid = np.full(N_PAD, -1, dtype=np.int64)
    old_of_fid[fid_of_old] = np.arange(N)

    nsrc = fid_of_old[src]
    ndst = fid_of_old[dst]
    ndeg = np.zeros(N_PAD, dtype=np.int64)
    ndeg[fid_of_old] = deg

    K_ct = ndeg.reshape(NCORES, T_PER_CORE, P).max(-1)
    K_t = np.maximum(K_ct.max(0), 1)[:T_DATA]               # [98]
    tot_slabs = int(K_t.sum())
    slab_off = np.concatenate([[0], np.cumsum(K_t)]).astype(np.int64)

    # slot index k of each edge within its dst; self-loop forced to k=0
    notself = (nsrc != ndst).astype(np.int8)
    key = np.lexsort((notself, ndst))
    ds_ = ndst[key]
    ss_ = nsrc[key]
    counts = np.bincount(ds_, minlength=N_PAD)
    run_start = np.zeros(N_PAD + 1, dtype=np.int64)
    run_start[1:] = np.cumsum(counts)
    kk = np.arange(ds_.size, dtype=np.int64) - run_start[ds_]

    blk = ds_ // P
    core = blk // T_PER_CORE
    t_idx = blk % T_PER_CORE
    d_rel = ds_ % P
    col = slab_off[t_idx] + kk

    # per-core slab tables: old x-row id of src (N = dummy), src fid (-1 = dummy)
    src_old = np.full((NCORES, P, tot_slabs), N, dtype=np.int64)
    src_old[core, d_rel, col] = old_of_fid[ss_]
    src_fid = np.full((NCORES, P, tot_slabs), -1, dtype=np.int64)
    src_fid[core, d_rel, col] = ss_

    # dummy x row: x_d @ (W1 @ A1s) == -1e4 for all 4 heads (min-norm lstsq)
    A1s = np.zeros((HID, HEADS), dtype=np.float64)
    A1d = np.zeros((HID, HEADS), dtype=np.float64)
    for h in range(HEADS):
        A1s[h * C1:(h + 1) * C1, h] = np.asarray(att_src1, np.float64)[h]
        A1d[h * C1:(h + 1) * C1, h] = np.asarray(att_dst1, np.float64)[h]
    W1_ = np.asarray(W1, np.float64)
    M = W1_ @ A1s                                           # [256, 4]
    x_dummy, *_ = np.linalg.lstsq(M.T, np.full(HEADS, PAD_ALPHA), rcond=None)

    x_pad = np.concatenate(
        [np.asarray(x, np.float32), x_dummy[None, :].astype(np.float32)], axis=0
    ).astype(bf16)                                          # [N+1, 256]

    # per-core xe: [128, tot_slabs * 2 * 128] bf16, xe[f, (s,i,d)] =
    # x_pad[src_old[c, d, s], i*128 + f]
    xe_cores = []
    for c in range(NCORES):
        srcs_T = src_old[c].T                               # [S, 128]
        xv = x_pad[srcs_T]                                  # [S, 128d, 256f]
        arr = np.ascontiguousarray(xv.transpose(2, 0, 1))   # [256, S, 128]
        xe = np.ascontiguousarray(
            arr.reshape(2, P, tot_slabs, P).transpose(1, 2, 0, 3)
        ).reshape(P, tot_slabs * 2 * P)
        xe_cores.append(xe)

    # phase-3 idx (wrapped int16, replicated in all 8 16-partition groups)
    # + bias mask [128, tot_slabs*16] bf16
    g16 = np.where(src_fid >= 0, src_fid >> 4, 0).astype(np.int16)   # [NC,128,S]
    sub = np.where(src_fid >= 0, src_fid & 15, -1)                   # [NC,128,S]
    idx3_cores = []
    bias_cores = []
    chunks = []                       # (tile, k0, k1, idx_col_off) shared
    icol = 0
    for t in range(T_DATA):
        K = int(K_t[t])
        off = int(slab_off[t])
        for k0 in range(1, K, CHUNK):
            k1 = min(k0 + CHUNK, K)
            chunks.append((t, k0, k1, icol))
            icol += (k1 - k0) * P // 16
    idx_cols_total = icol
    for c in range(NCORES):
        idx3 = np.zeros((P, idx_cols_total), dtype=np.int16)
        for (t, k0, k1, ic) in chunks:
            off = int(slab_off[t])
            flat = g16[c][:, off + k0:off + k1].T.reshape(-1)  # j=(k_rel)*128+d
            blkw = flat.reshape(-1, 16).T                      # [16, n/16]
            idx3[:, ic:ic + blkw.shape[1]] = np.tile(blkw, (8, 1))
        idx3_cores.append(idx3)
        b3 = np.full((P, tot_slabs, 16), PAD_ALPHA, dtype=np.float32)
        valid = sub[c] >= 0
        dd, ss2 = np.nonzero(valid)
        b3[dd, ss2, sub[c][dd, ss2]] = 0.0
        bias_cores.append(b3.reshape(P, tot_slabs * 16).astype(bf16))

    W1aug = np.concatenate(
        [W1_, W1_ @ A1s, W1_ @ A1d], axis=1).astype(np.float32)    # [256,136]
    W1aug_t = np.ascontiguousarray(
        W1aug.reshape(2, 128, D1).transpose(1, 0, 2)).astype(bf16)

    W2_ = np.asarray(W2, np.float32)
    a_s2 = np.asarray(att_src2, np.float32).reshape(OUT, 1)
    a_d2 = np.asarray(att_dst2, np.float32).reshape(OUT, 1)
    W2aug = np.concatenate([W2_, W2_ @ a_s2, W2_ @ a_d2], axis=1).astype(bf16)

    b1_b = np.tile(np.asarray(b1, np.float32)[None, :], (P, 1))
    b2_b = np.tile(np.asarray(b2, np.float32)[None, :], (P, 1))

    return dict(
        xe_cores=xe_cores, idx3_cores=idx3_cores, bias_cores=bias_cores,
        W1aug_t=W1aug_t, W2aug=W2aug, b1_b=b1_b, b2_b=b2_b,
        K_t=K_t, slab_off=slab_off, tot_slabs=tot_slabs, chunks=chunks,
        idx_cols_total=idx_cols_total, old_of_fid=old_of_fid,
    )


def _build_program(K_t, slab_off, tot_slabs, chunks, idx_cols_total,
                   dump_tables=False):
    import concourse.bass as bass
    import concourse.mybir as mybir
    import bass_rust as _bass_rust
    from concourse import library_config
    from concourse.tile import TileContext
    from concourse.masks import make_identity

    f32 = mybir.dt.float32
    bf16 = mybir.dt.bfloat16
    i16 = mybir.dt.int16
    AF = mybir.ActivationFunctionType
    OP = mybir.AluOpType

    nc = bass.Bass(target_bir_lowering=False)

    xe_in = nc.dram_tensor("xe", [P, tot_slabs * 2 * P], bf16, kind="ExternalInput")
    idx3_in = nc.dram_tensor("idx3", [P, idx_cols_total], i16, kind="ExternalInput")
    bias_in = nc.dram_tensor("bias3", [P, tot_slabs * 16], bf16, kind="ExternalInput")
    w1aug = nc.dram_tensor("w1aug", [P, 2, D1], bf16, kind="ExternalInput")
    w2aug = nc.dram_tensor("w2aug", [HID, 4], bf16, kind="ExternalInput")
    b1_b = nc.dram_tensor("b1_b", [P, HID], f32, kind="ExternalInput")
    b2_b = nc.dram_tensor("b2_b", [P, OUT], f32, kind="ExternalInput")
    out2 = nc.dram_tensor("out2", [ROWS_PER_CORE, OUT], f32, kind="ExternalOutput")
    if dump_tables:
        t2dump = nc.dram_tensor("t2dump", [N_PAD, 4], f32, kind="ExternalOutput")

    with TileContext(nc) as tc:
        with tc.tile_pool(name="dram", bufs=1, space="DRAM") as dpool, \
             tc.tile_pool(name="const", bufs=1) as cpool, \
             tc.tile_pool(name="xp", bufs=2) as xp, \
             tc.tile_pool(name="gat", bufs=2) as gat, \
             tc.tile_pool(name="sb", bufs=4) as sb, \
             tc.tile_pool(name="acc", bufs=2) as accp, \
             tc.tile_pool(name="g3", bufs=5) as g3p, \
             tc.tile_pool(name="ps", bufs=4, space="PSUM") as ps, \
             tc.tile_pool(name="ps2", bufs=2, space="PSUM") as ps2:

            t2l = dpool.tile([ROWS_PER_CORE, 4], f32)
            table2 = dpool.tile([N_PAD, 4], f32, addr_space="Shared")

            w1_sb = cpool.tile([P, 2, D1], bf16)
            nc.sync.dma_start(out=w1_sb[:], in_=w1aug[:, :, :])
            w2_sb = cpool.tile([HID, 4], bf16)
            nc.sync.dma_start(out=w2_sb[:], in_=w2aug[:, :])
            b1_sb = cpool.tile([P, HID], f32)
            nc.sync.dma_start(out=b1_sb[:], in_=b1_b[:, :])
            b2_sb = cpool.tile([P, OUT], f32)
            nc.sync.dma_start(out=b2_sb[:], in_=b2_b[:, :])
            ident = cpool.tile([P, P], bf16)
            make_identity(nc, ident[:])
            ix3 = cpool.tile([P, idx_cols_total], i16)
            nc.sync.dma_start(out=ix3[:], in_=idx3_in[:, :])
            t2_all = cpool.tile([P, T_DATA, 4], f32)

            # ---- Phase A: fused layer-1 per dst tile ----
            for t in range(T_DATA):
                K = int(K_t[t])
                off = int(slab_off[t])
                xt = xp.tile([P, K, 2, P], bf16, tag="xt")
                nc.sync.dma_start(
                    out=xt[:], in_=xe_in[:, off * 2 * P:(off + K) * 2 * P]
                    .rearrange("p (k i d) -> p k i d", i=2, d=P))
                gall = gat.tile([P, K, D1], bf16, tag="gall")
                for k in range(K):
                    hp = ps.tile([P, D1], f32, space="PSUM", tag="hp")
                    for i in range(2):
                        nc.tensor.matmul(out=hp[:], lhsT=xt[:, k, i, :],
                                         rhs=w1_sb[:, i, :],
                                         start=(i == 0), stop=(i == 1))
                    nc.scalar.activation(gall[:, k, :], hp[:], AF.Copy)
                # e[p, h, k] = a_src(slab k) + a_dst(slab 0 self-loop)
                w_all = sb.tile([P, HEADS, K], f32, tag="w_all")
                nc.vector.tensor_tensor(
                    out=w_all[:],
                    in0=gall[:, :, HID:HID + HEADS].rearrange("p k h -> p h k"),
                    in1=gall[:, 0, HID + HEADS:HID + 2 * HEADS].to_broadcast(
                        [P, HEADS, K]),
                    op=OP.add)
                nc.scalar.activation(w_all[:], w_all[:], AF.Prelu, alpha=NEG)
                w16 = sb.tile([P, HEADS, K], bf16, tag="w16")
                nc.scalar.activation(w16[:], w_all[:], AF.Exp)
                den = sb.tile([P, HEADS], f32, tag="den")
                nc.vector.tensor_reduce(out=den[:], in_=w16[:],
                                        axis=mybir.AxisListType.X, op=OP.add)
                rden = sb.tile([P, HEADS], f32, tag="rden")
                nc.vector.tensor_scalar(out=rden[:], in0=den[:], scalar1=1e-30,
                                        scalar2=None, op0=OP.add)
                nc.vector.reciprocal(rden[:], rden[:])
                u_all = accp.tile([P, K, HID], bf16, tag="u_all")
                nc.vector.tensor_tensor(
                    out=u_all[:].rearrange("p k (h c) -> p k h c", h=HEADS),
                    in0=gall[:, :, 0:HID].rearrange("p k (h c) -> p k h c", h=HEADS),
                    in1=w16[:].rearrange("p h k -> p k h").to_broadcast(
                        [P, K, HEADS, C1]),
                    op=OP.mult)
                acc = sb.tile([P, HID], f32, tag="acc")
                nc.vector.tensor_reduce(
                    out=acc[:], in_=u_all[:].rearrange("p k f -> p f k"),
                    axis=mybir.AxisListType.X, op=OP.add)
                h1 = sb.tile([P, HID], f32, tag="h1")
                nc.vector.tensor_tensor(
                    out=h1[:].rearrange("p (h c) -> p h c", h=HEADS),
                    in0=acc[:].rearrange("p (h c) -> p h c", h=HEADS),
                    in1=rden[:].to_broadcast([P, HEADS, C1]),
                    op=OP.mult)
                nc.vector.tensor_add(h1[:], h1[:], b1_sb[:])
                # elu
                m = sb.tile([P, HID], f32, tag="m")
                nc.vector.tensor_scalar(out=m[:], in0=h1[:], scalar1=0.0,
                                        scalar2=None, op0=OP.min)
                em = sb.tile([P, HID], f32, tag="em")
                nc.scalar.activation(em[:], m[:], AF.Exp)
                nc.vector.tensor_scalar(out=h1[:], in0=h1[:], scalar1=0.0,
                                        scalar2=None, op0=OP.max)
                nc.vector.tensor_add(h1[:], h1[:], em[:])
                h1b = sb.tile([P, HID], bf16, tag="h1b")
                nc.vector.tensor_scalar(out=h1b[:], in0=h1[:], scalar1=1.0,
                                        scalar2=None, op0=OP.subtract)
                # t2 row block = h1 @ W2aug (via PE transpose)
                h1tp = ps2.tile([P, P], bf16, space="PSUM", tag="h1tp")
                nc.tensor.transpose(out=h1tp[:], in_=h1b[:], identity=ident[:])
                h1t = sb.tile([P, P], bf16, tag="h1t")
                nc.vector.tensor_copy(h1t[:], h1tp[:])
                t2p = ps2.tile([P, 4], f32, space="PSUM", tag="t2p")
                nc.tensor.matmul(out=t2p[:], lhsT=h1t[:], rhs=w2_sb[:],
                                 start=True, stop=True)
                t2 = sb.tile([P, 4], f32, tag="t2")
                nc.vector.tensor_copy(t2[:], t2p[:])
                nc.vector.tensor_copy(t2_all[:, t, :], t2[:])
                nc.scalar.dma_start(out=t2l[t * P:(t + 1) * P, :], in_=t2[:])

            # ---- Phase B: AllGather compact table2 ----
            nc.gpsimd.collective_compute(
                "AllGather", mybir.AluOpType.bypass,
                replica_groups=[list(range(NCORES))],
                ins=[t2l[:, :]], outs=[table2[:, :]])

            table16 = table2[:, :].rearrange("(a b) f -> a (b f)", b=16)

            regs = {}
            def reg_of(v):
                if v not in regs:
                    regs[v] = nc.gpsimd.to_reg(v)
                return regs[v]

            # ---- Phase C: layer-2 per dst tile ----
            gcount = 0
            cur = 0
            for t in range(T_DATA):
                K = int(K_t[t])
                off = int(slab_off[t])
                K3 = K - 1
                g3 = g3p.tile([P, K3, 64], f32, tag="g3")
                while cur < len(chunks) and chunks[cur][0] == t:
                    _, k0, k1, ic = chunks[cur]
                    n_idx = (k1 - k0) * P
                    nc.gpsimd.dma_gather(
                        g3[:, k0 - 1:k1 - 1, :], table16,
                        ix3[:, ic:ic + n_idx // 16],
                        n_idx, reg_of(n_idx), 64)
                    gcount += 1
                    if gcount % DRAIN_EVERY == 0:
                        nc.gpsimd.drain()
                    cur += 1
                bias = sb.tile([P, K3 * 16], bf16, tag="bias3")
                nc.sync.dma_start(out=bias[:],
                                  in_=bias_in[:, (off + 1) * 16:(off + K) * 16])
                e3 = sb.tile([P, K3 * 16], f32, tag="e3")
                nc.vector.tensor_tensor(
                    out=e3[:],
                    in0=g3[:].rearrange("p k (q v) -> p (k q) v", v=4)[:, :, 2]
                    .rearrange("p m -> p m"),
                    in1=bias[:],
                    op=OP.add)
                nc.vector.tensor_tensor(
                    out=e3[:], in0=e3[:],
                    in1=t2_all[:, t, 3:4].to_broadcast([P, K3 * 16]),
                    op=OP.add)
                nc.scalar.activation(e3[:], e3[:], AF.Prelu, alpha=NEG)
                nc.scalar.activation(e3[:], e3[:], AF.Exp)
                den3 = sb.tile([P, 1], f32, tag="den3")
                nc.vector.tensor_reduce(out=den3[:], in_=e3[:],
                                        axis=mybir.AxisListType.X, op=OP.add)
                ws = sb.tile([P, 1], f32, tag="ws")
                nc.vector.tensor_tensor(out=ws[:], in0=t2_all[:, t, 2:3],
                                        in1=t2_all[:, t, 3:4], op=OP.add)
                nc.scalar.activation(ws[:], ws[:], AF.Prelu, alpha=NEG)
                nc.scalar.activation(ws[:], ws[:], AF.Exp)
                nc.vector.tensor_tensor(out=den3[:], in0=den3[:], in1=ws[:],
                                        op=OP.add)
                acc3 = sb.tile([P, OUT], f32, tag="acc3")
                u3 = sb.tile([P, K3 * 16], f32, tag="u3")
                for o in range(OUT):
                    nc.vector.tensor_tensor(
                        out=u3[:],
                        in0=g3[:].rearrange("p k (q v) -> p (k q) v", v=4)[:, :, o]
                        .rearrange("p m -> p m"),
                        in1=e3[:], op=OP.mult)
                    nc.vector.tensor_reduce(out=acc3[:, o:o + 1], in_=u3[:],
                                            axis=mybir.AxisListType.X, op=OP.add)
                wh = sb.tile([P, OUT], f32, tag="wh")
                nc.vector.tensor_tensor(out=wh[:], in0=t2_all[:, t, 0:2],
                                        in1=ws[:].to_broadcast([P, OUT]),
                                        op=OP.mult)
                nc.vector.tensor_add(acc3[:], acc3[:], wh[:])
                rden3 = sb.tile([P, 1], f32, tag="rden3")
                nc.vector.tensor_scalar(out=rden3[:], in0=den3[:], scalar1=1e-30,
                                        scalar2=None, op0=OP.add)
                nc.vector.reciprocal(rden3[:], rden3[:])
                o2 = sb.tile([P, OUT], f32, tag="o2")
                nc.vector.tensor_tensor(out=o2[:], in0=acc3[:],
                                        in1=rden3[:].to_broadcast([P, OUT]),
                                        op=OP.mult)
                nc.vector.tensor_add(o2[:], o2[:], b2_sb[:])
                nc.sync.dma_start(out=out2[t * P:(t + 1) * P, :], in_=o2[:])

            if dump_tables:
                for r0 in range(0, N_PAD, 16896):
                    r1 = min(r0 + 16896, N_PAD)
                    nc.sync.dma_start(out=t2dump[r0:r1, :], in_=table2[r0:r1, :])

    # lower library loads + custom isa, then fix multi-wait instructions
    inst_type_to_lib_mask = {}
    for lib in library_config.all_libraries:
        for it in lib.instructions:
            inst_type_to_lib_mask[it] = inst_type_to_lib_mask.get(it, 0) | (
                1 << lib.index)
    _bass_rust.insert_library_loads(nc, inst_type_to_lib_mask,
                                    len(library_config.all_libraries),
                                    library_config.standard.index)
    mybir.codegen_inst_isa_subclasses(nc)
    _split_waits(nc)
    return nc


def _split_waits(nc, max_waits=1):
    """This walrus build allows one sync-wait slot per instruction; hoist
    excess waits onto same-engine InstNoOp carriers inserted just before."""
    import concourse.mybir as mybir
    ctr = 0
    for fn in nc.m.functions:
        for bb in fn.blocks:
            out = []
            changed = False
            for inst in bb.instructions:
                si = inst.sync_info
                if si is not None and len(si.on_wait) > max_waits:
                    waits = list(si.on_wait)
                    extra, keep = waits[:-max_waits], waits[-max_waits:]
                    for i in range(0, len(extra), max_waits):
                        ctr += 1
                        nop = mybir.InstNoOp(name=f"waitfix-{ctr}", ins=[], outs=[])
                        nop.engine = inst.engine
                        nop.sync_info = mybir.SyncInfo(
                            on_wait=extra[i:i + max_waits], on_update=[])
                        out.append(nop)
                    si.on_wait = keep
                    inst.sync_info = si
                    changed = True
                out.append(inst)
            if changed:
                bb.instructions = out
    return ctr


def kernel(x, edge_index, W1, att_src1, att_dst1, b1, W2, att_src2, att_dst2, b2):
    from concourse.bass_utils import run_bass_kernel_spmd

    prep = _host_prep(x, edge_index, W1, att_src1, att_dst1, b1,
                      W2, att_src2, att_dst2, b2)
    dump = bool(os.environ.get("GAT_DUMP"))
    nc = _build_program(prep["K_t"], prep["slab_off"], prep["tot_slabs"],
                        prep["chunks"], prep["idx_cols_total"],
                        dump_tables=dump)

    in_maps = []
    for c in range(NCORES):
        in_maps.append({
            "xe": prep["xe_cores"][c],
            "idx3": prep["idx3_cores"][c],
            "bias3": prep["bias_cores"][c],
            "w1aug": prep["W1aug_t"],
            "w2aug": prep["W2aug"],
            "b1_b": prep["b1_b"],
            "b2_b": prep["b2_b"],
        })

    trace = bool(os.environ.get("GAT_TRACE"))
    if trace:
        _install_ntff_shim()
    r = run_bass_kernel_spmd(nc, in_maps, core_ids=list(range(NCORES)),
                             trace=trace)
    if trace and r.exec_time_ns:
        print(f"HW exec time: {r.exec_time_ns} ns")

    old_of_fid = prep["old_of_fid"]
    out = np.zeros((N, OUT), dtype=np.float32)
    for c in range(NCORES):
        fid0 = c * ROWS_PER_CORE
        olds = old_of_fid[fid0:fid0 + ROWS_PER_CORE]
        m = olds >= 0
        out[olds[m]] = r.results[c]["out2"][m]
    if dump:
        np.save("/tmp/gat_t2dump.npy", r.results[0]["t2dump"])
        np.save("/tmp/gat_oldfid.npy", old_of_fid)
    return out


def _install_ntff_shim():
    """The image's antenv lacks axon_hooks; recreate it so trace=True works."""
    import sys, types
    if "antenv.axon_hooks" in sys.modules:
        return
    sys.path.insert(0, "/root/.axon_site/trn_agent_boot")
    try:
        import trn_boot
        hook = trn_boot._ntff_profile_via_ctypes("/opt/axon/libaxon_pjrt.so")
    except Exception:
        hook = None
    mod = types.ModuleType("antenv.axon_hooks")
    mod.get_axon_ntff_profile_hook = lambda: hook
    mod.set_axon_ntff_profile_hook = lambda h: None
    sys.modules["antenv.axon_hooks"] = mod
